# revision 17
# baseline (speedup 1.0000x reference)
"""Grok1-style attention on 8 trn2 NeuronCores, tensor-parallel over heads.

Sharding (per core c of 8):
  - q heads 4c..4c+3 (512 features), kv head c (128+128 features)
  - w_qkv sharded column-wise (by head), w_o row-wise; partial o_proj
    outputs summed on host (the all-reduce).

Device layout trick: qkv is computed TRANSPOSED (features on partitions,
positions on free axis), so scores (k^T q), probs*V and o_proj chain with
no transposes except 16 cheap PE transposes of V.

Softmax: tanh logit-cap bounds scores to +-30 so exp() cannot overflow ->
no row-max pass. Row sums: exp tiles are accumulated into an fp32 SBUF
accumulator (DVE/Pool alternating), then a single ones-vector matmul per
(head, q-tile) contracts the 128 partitions; normalization via a rank-1
broadcast matmul + reciprocal_approx_fast.

Precision: bf16 matmul operands (PE runs fp32 4x slower), fp32 PSUM
accumulation, tanh kept in fp32. All HBM traffic is bf16 (host pre-casts
inputs, partial outputs summed on host in fp32).
"""
import numpy as np
import ml_dtypes
from contextlib import ExitStack

import concourse.bass as bass
import concourse.mybir as mybir
import concourse.tile as tile
from concourse import bacc
from concourse.bass_utils import run_bass_kernel_spmd
from concourse.masks import make_identity

T = 2048
D = 4096
HD = 128
HALF = 64
NCORES = 8
HPC = 4                    # q heads per core
QF = HPC * HD              # 512
NF = QF + 2 * HD           # 768 qkv features per core
NCH = D // 128             # 32 contraction chunks
CH = 4                     # contraction chunks per h dma
NHC = NCH // CH            # 8 h dmas per t-tile
WQC = 8                    # contraction chunks per wq dma
TT = 512                   # t-tile width (matmul moving dim)
NTT = T // TT              # 4
NKT = T // 128             # 16 k-tiles
SCALING = HD ** -0.5
CAP = 30.0
BF = mybir.dt.bfloat16
F32 = mybir.dt.float32


def _emit(nc):
    hT = nc.dram_tensor("hT", [D, T], BF, kind="ExternalInput").ap()
    wq = nc.dram_tensor("wq", [D, NF], BF, kind="ExternalInput").ap()
    wo = nc.dram_tensor("wo", [QF, D], BF, kind="ExternalInput").ap()
    cc = nc.dram_tensor("cc", [HD, T], BF, kind="ExternalInput").ap()
    ss = nc.dram_tensor("ss", [HD, T], BF, kind="ExternalInput").ap()
    mk = nc.dram_tensor("mk", [4, 128, TT], BF, kind="ExternalInput").ap()
    out = nc.dram_tensor("out", [T, D], BF, kind="ExternalOutput").ap()

    with tile.TileContext(nc) as tc:
        with ExitStack() as ctx:
            wqp = ctx.enter_context(tc.tile_pool(name="wqp", bufs=1))
            wop = ctx.enter_context(tc.tile_pool(name="wop", bufs=1))
            hp = ctx.enter_context(tc.tile_pool(name="hp", bufs=8))
            cstp = ctx.enter_context(tc.tile_pool(name="cstp", bufs=1))
            seqp = ctx.enter_context(tc.tile_pool(name="seqp", bufs=1))
            rtp = ctx.enter_context(tc.tile_pool(name="rtp", bufs=2))
            stp = ctx.enter_context(tc.tile_pool(name="stp", bufs=2))
            etp = ctx.enter_context(tc.tile_pool(name="etp", bufs=3))
            eap = ctx.enter_context(tc.tile_pool(name="eap", bufs=2))
            smp = ctx.enter_context(tc.tile_pool(name="smp", bufs=2))
            obp = ctx.enter_context(tc.tile_pool(name="obp", bufs=2))
            psp = ctx.enter_context(tc.tile_pool(name="psp", bufs=1, space="PSUM"))

            # ---- resident loads: wq chunked by (i, fg-half) so the first
            # matmul only waits for 0.79 MB; stagger the rest ----
            wq_r = wq.rearrange("(i p) f -> p i f", p=128)     # i = 32 chunks
            wq4 = []
            for i in range(NCH // WQC):
                wa = wqp.tile([128, WQC, 384], BF, tag=f"wq{i}a", name=f"wq{i}a")
                wb = wqp.tile([128, WQC, 384], BF, tag=f"wq{i}b", name=f"wq{i}b")
                wq4.append((wa, wb))
            nc.gpsimd.dma_start(wq4[0][0][:], wq_r[:, 0:WQC, 0:384])
            cc_sb = cstp.tile([HD, T], BF, tag="cc")
            ss_sb = cstp.tile([HD, T], BF, tag="ss")
            mk_sb = cstp.tile([128, 4, TT], BF, tag="mk")
            # stagger the non-critical loads so they don't steal HBM
            # bandwidth from wq chunk 0 / the first h chunks
            with tc.tile_wait_until(0.004):
                nc.gpsimd.dma_start(cc_sb[:], cc[:, :])
                nc.gpsimd.dma_start(ss_sb[:], ss[:, :])
            with tc.tile_wait_until(0.006):
                nc.gpsimd.dma_start(wq4[1][0][:], wq_r[:, WQC:2 * WQC, 0:384])
            with tc.tile_wait_until(0.009):
                nc.gpsimd.dma_start(wq4[2][0][:], wq_r[:, 2 * WQC:3 * WQC, 0:384])
                nc.gpsimd.dma_start(mk_sb[:], mk.rearrange("m p t -> p m t"))
            with tc.tile_wait_until(0.012):
                nc.gpsimd.dma_start(wq4[3][0][:], wq_r[:, 3 * WQC:4 * WQC, 0:384])
            with tc.tile_wait_until(0.015):
                for i in range(NCH // WQC):
                    nc.gpsimd.dma_start(
                        wq4[i][1][:], wq_r[:, i * WQC:(i + 1) * WQC, 384:NF])
            wo_r = wo.rearrange("(c p) n -> p c n", p=128)
            wo_t = []
            with tc.tile_wait_until(0.028):
                for j in range(2):
                    w_j = wop.tile([128, 2, D], BF, tag=f"wo{j}", name=f"wo{j}")
                    nc.gpsimd.dma_start(w_j[:], wo_r[:, 2 * j:2 * j + 2, :])
                    wo_t.append(w_j)

            # PE warm-up: ~60 tiny matmuls fill the HAM activity window
            # while the first weight/activation DMAs are in flight, so the
            # first real matmuls run at 2.4 GHz instead of 1.2
            warm = cstp.tile([1, 64], BF, tag="warm")
            nc.gpsimd.memset(warm[:], 0.0)
            wps = psp.tile([64, 64], F32, tag="b7", name="warm_ps")
            for _ in range(60):
                nc.tensor.matmul(wps[:, :], warm[:], warm[:], start=True, stop=True)
            ident = cstp.tile([128, 128], BF, tag="id")
            make_identity(nc, ident[:])
            ones_k = cstp.tile([128, 1], BF, tag="ones_k")
            nc.gpsimd.memset(ones_k[:], 1.0)
            ones_m = cstp.tile([1, 128], BF, tag="ones_m")
            nc.gpsimd.memset(ones_m[:], 1.0)

            # per-t-tile tiles so later phases can start early
            qTt = [[seqp.tile([HD, TT], BF, tag=f"q{h}_{tt}", name=f"qT{h}_{tt}")
                    for tt in range(NTT)] for h in range(HPC)]
            kTt = [seqp.tile([HD, TT], BF, tag=f"k_{tt}", name=f"kT{tt}")
                   for tt in range(NTT)]
            vbt = [seqp.tile([128, HD], BF, tag=f"vb_{kt}", name=f"vb{kt}")
                   for kt in range(NKT)]
            atq = [[seqp.tile([HD, TT], BF, tag=f"a{h}_{qt}", name=f"at{h}_{qt}")
                    for qt in range(NTT)] for h in range(HPC)]

            # ---- phase 1: qkv projection (transposed) + rope ----
            # psum banks: b0-b2 qkv accum, b3 transposes,
            #             b4-b5 scores, b6 attn accum, b7 denom+bcast
            hT_r = hT.rearrange("(c p) t -> p c t", p=128)
            for tt in range(NTT):
                t0 = tt * TT
                hcs = []
                for hc in range(NHC):
                    h_t = hp.tile([128, CH, TT], BF, tag="h", name=f"h{tt}_{hc}")
                    with tc.tile_wait_until(
                            0.003 + 0.0012 * hc, enable=(tt == 0 and hc >= 1)):
                        nc.sync.dma_start(
                            h_t[:], hT_r[:, hc * CH:(hc + 1) * CH, t0:t0 + TT])
                    hcs.append(h_t)
                vT_t = rtp.tile([HD, TT], BF, tag="vT", name=f"vT{tt}")
                c_t = cc_sb[:, t0:t0 + TT]
                s_t = ss_sb[:, t0:t0 + TT]
                for fg in range(2):
                    ps3 = [psp.tile([128, TT], F32, tag=f"b{j}", name=f"qkv_ps{j}")
                           for j in range(3)]
                    for c in range(NCH):
                        src = hcs[c // CH][:, c % CH, :]
                        for j in range(3):
                            f = fg * 3 + j
                            nc.tensor.matmul(
                                ps3[j][:],
                                wq4[c // WQC][fg][:, c % WQC, j * 128:(j + 1) * 128],
                                src,
                                start=(c == 0),
                                stop=(c == NCH - 1),
                            )
                    for j in range(3):
                        f = fg * 3 + j
                        if f < 5:
                            dst = qTt[f][tt] if f < HPC else kTt[tt]
                            qk_sb = rtp.tile([128, TT], BF, tag="qk_sb")
                            nc.vector.tensor_copy(qk_sb[:], ps3[j][:])
                            # rotated copy: [x2; x1] via partition-swap DMA
                            rot = rtp.tile([128, TT], BF, tag="rot")
                            nc.scalar.dma_start(rot[0:HALF, :], qk_sb[HALF:128, :])
                            nc.scalar.dma_start(rot[HALF:128, :], qk_sb[0:HALF, :])
                            m1 = rtp.tile([128, TT], BF, tag="m1")
                            nc.vector.tensor_mul(m1[:], qk_sb[:], c_t)
                            m2 = rtp.tile([128, TT], BF, tag="m2")
                            nc.vector.tensor_mul(m2[:], rot[:], s_t)
                            nc.vector.tensor_add(dst[:], m1[:], m2[:])
                        else:
                            nc.vector.tensor_copy(vT_t[:], ps3[j][:])
                # transpose this t-tile's V to [t, d] blocks (DMA XBAR)
                for i in range(4):
                    kt = 4 * tt + i
                    nc.sync.dma_start(
                        vbt[kt][:], vT_t[:, i * 128:(i + 1) * 128], transpose=True)

            # ---- phase 2+3 interleaved: attention (h,qt) with o_proj
            # groups of qt-1 as PE filler while ACT catches up ----
            def oproj_group(t16, half, tail=False):
                t0o = t16 * 128
                ob = obp.tile([128, 4, TT], BF, tag="ob", name=f"ob{t16}_{half}")
                for n in range(4):
                    pls_n = psp.tile([128, TT], F32, tag=f"b{n}", name=f"o_ps{n}")
                    n0 = (half * 4 + n) * TT
                    for fc in range(HPC):
                        lhsT = atq[fc][t16 // 4][:, (t16 % 4) * 128:(t16 % 4 + 1) * 128]
                        nc.tensor.matmul(
                            pls_n[:], lhsT, wo_t[fc // 2][:, fc % 2, n0:n0 + TT],
                            start=(fc == 0), stop=(fc == HPC - 1),
                        )
                    if tail and n >= 2:
                        nc.scalar.copy(ob[:, n, :], pls_n[:])
                    else:
                        nc.vector.tensor_copy(ob[:, n, :], pls_n[:])
                nc.gpsimd.dma_start(
                    out[t0o:t0o + 128, half * 2048:(half + 1) * 2048]
                    .rearrange("p (n t) -> p n t", n=4),
                    ob[:],
                )

            for qt in range(NTT):
                for h in range(HPC):
                    a_ps = psp.tile([HD, TT], F32, tag="b6", name="a_ps")
                    e_acc = eap.tile([128, TT], F32, tag="ea", name=f"ea{h}_{qt}")
                    nkt = 4 * qt + 4
                    for kt in range(nkt):
                        m = kt - 4 * qt
                        j0 = 128 * m if m >= 0 else 0  # skip fully-masked cols
                        s_ps = psp.tile([128, TT], F32, tag=f"b{4 + kt % 2}", name="s_ps")
                        nc.tensor.matmul(
                            s_ps[:, j0:TT], kTt[kt // 4][:, (kt % 4) * 128:(kt % 4 + 1) * 128],
                            qTt[h][qt][:, j0:TT],
                            start=True, stop=True,
                        )
                        st = stp.tile([128, TT], F32, tag="st")
                        nc.scalar.activation(
                            st[:, j0:TT], s_ps[:, j0:TT],
                            mybir.ActivationFunctionType.Tanh,
                            scale=SCALING / CAP,
                        )
                        et = etp.tile([128, TT], BF, tag="et")
                        nc.scalar.activation(
                            et[:, j0:TT], st[:, j0:TT],
                            mybir.ActivationFunctionType.Exp,
                            scale=CAP,
                        )
                        if m >= 0:
                            # causal mask: zero where k0+i > q0+j
                            nc.gpsimd.tensor_mul(
                                et[:, j0:TT], et[:, j0:TT], mk_sb[:, m, j0:TT])
                        nc.tensor.matmul(
                            a_ps[:, j0:TT], vbt[kt][:], et[:, j0:TT],
                            start=(kt == 0), stop=(kt == nkt - 1),
                        )
                        # fp32 accumulation of exp tiles for the denominator
                        if kt == 0:
                            nc.vector.tensor_copy(e_acc[:], et[:])
                        else:
                            nc.vector.tensor_add(
                                e_acc[:, j0:TT], e_acc[:, j0:TT], et[:, j0:TT])
                    e_bf = smp.tile([128, TT], BF, tag="ebf")
                    nc.vector.tensor_copy(e_bf[:], e_acc[:])
                    d_ps = psp.tile([1, TT], F32, tag="b7", name="d_ps")
                    nc.tensor.matmul(d_ps[:], ones_k[:], e_bf[:], start=True, stop=True)
                    rc = smp.tile([1, TT], F32, tag="rc")
                    nc.vector.reciprocal_approx_fast(out=rc[:], in_=d_ps[:])
                    rcb = smp.tile([1, TT], BF, tag="rcb")
                    nc.vector.tensor_copy(rcb[:], rc[:])
                    bc_ps = psp.tile([128, TT], F32, tag="b7", name="bc_ps")
                    nc.tensor.matmul(bc_ps[:], ones_m[:], rcb[:], start=True, stop=True)
                    bc_sb = smp.tile([128, TT], F32, tag="bcs")
                    nc.vector.tensor_copy(bc_sb[:], bc_ps[:])
                    nc.vector.tensor_mul(atq[h][qt][:], a_ps[:], bc_sb[:])
                    # PE filler: o_proj groups of the previous qt
                    if qt >= 1:
                        for k in range(2):
                            gi = h * 2 + k
                            oproj_group(4 * (qt - 1) + gi // 2, gi % 2,
                                        tail=(qt == 3))
            for gi in range(8):
                oproj_group(12 + gi // 2, gi % 2, tail=True)
    return nc

_CACHE = {}


def _get_nc():
    if "nc" not in _CACHE:
        nc = bacc.Bacc("TRN2", target_bir_lowering=False, debug=False)
        _emit(nc)
        nc.compile()
        _CACHE["nc"] = nc
    return _CACHE["nc"]


def _in_maps(positions, hidden_states, w_qkv, w_o):
    hidden_states = np.asarray(hidden_states, dtype=np.float32)
    w_qkv = np.asarray(w_qkv, dtype=np.float32)
    w_o = np.asarray(w_o, dtype=np.float32)
    pos = np.asarray(positions).astype(np.float64)
    bf = ml_dtypes.bfloat16

    hT = np.ascontiguousarray(hidden_states.T.astype(bf))
    inv_freq = 1.0 / (10000.0 ** (np.arange(HALF, dtype=np.float64) * 2.0 / HD))
    ang = np.outer(inv_freq, pos)                      # [64, T]
    cos = np.cos(ang).astype(np.float32)
    sin = np.sin(ang).astype(np.float32)
    cc = np.ascontiguousarray(np.concatenate([cos, cos], axis=0).astype(bf))
    ss = np.ascontiguousarray(np.concatenate([-sin, sin], axis=0).astype(bf))
    ii = np.arange(128)[:, None]
    jj = np.arange(TT)[None, :]
    mk = np.stack([(jj - ii - 128 * m >= 0) for m in range(4)]).astype(bf)

    in_maps = []
    for c in range(NCORES):
        rows = np.concatenate([
            w_qkv[QF * c:QF * (c + 1)],
            w_qkv[D + HD * c:D + HD * (c + 1)],
            w_qkv[D + HD * NCORES + HD * c:D + HD * NCORES + HD * (c + 1)],
        ], axis=0)                                      # [768, 4096]
        wq_c = np.ascontiguousarray(rows.T.astype(bf))  # [4096, 768]
        wo_c = np.ascontiguousarray(w_o[:, QF * c:QF * (c + 1)].T.astype(bf))
        in_maps.append({"hT": hT, "wq": wq_c, "wo": wo_c, "cc": cc, "ss": ss, "mk": mk})
    return in_maps


def run(positions, hidden_states, w_qkv, w_o, trace=False):
    nc = _get_nc()
    in_maps = _in_maps(positions, hidden_states, w_qkv, w_o)
    res = run_bass_kernel_spmd(nc, in_maps, list(range(NCORES)), trace=trace)
    parts = np.stack(
        [np.asarray(res.results[i]["out"]).astype(np.float32)
         for i in range(NCORES)], axis=0)
    full = parts.sum(axis=0, dtype=np.float64).astype(np.float32)
    return full, res


def kernel(positions, hidden_states, w_qkv, w_o):
    full, _ = run(positions, hidden_states, w_qkv, w_o, trace=False)
    return full


# revision 18
# speedup vs baseline: 1.0237x; 1.0237x over previous
"""Grok1-style attention on 8 trn2 NeuronCores, tensor-parallel over heads.

Sharding (per core c of 8):
  - q heads 4c..4c+3 (512 features), kv head c (128+128 features)
  - w_qkv sharded column-wise (by head), w_o row-wise; partial o_proj
    outputs summed on host (the all-reduce).

Device layout trick: qkv is computed TRANSPOSED (features on partitions,
positions on free axis), so scores (k^T q), probs*V and o_proj chain with
no transposes except 16 cheap PE transposes of V.

Softmax: tanh logit-cap bounds scores to +-30 so exp() cannot overflow ->
no row-max pass. Row sums: exp tiles are accumulated into an fp32 SBUF
accumulator (DVE/Pool alternating), then a single ones-vector matmul per
(head, q-tile) contracts the 128 partitions; normalization via a rank-1
broadcast matmul + reciprocal_approx_fast.

Precision: bf16 matmul operands (PE runs fp32 4x slower), fp32 PSUM
accumulation, tanh kept in fp32. All HBM traffic is bf16 (host pre-casts
inputs, partial outputs summed on host in fp32).
"""
import numpy as np
import ml_dtypes
from contextlib import ExitStack

import concourse.bass as bass
import concourse.mybir as mybir
import concourse.tile as tile
from concourse import bacc
from concourse.bass_utils import run_bass_kernel_spmd
from concourse.masks import make_identity

T = 2048
D = 4096
HD = 128
HALF = 64
NCORES = 8
HPC = 4                    # q heads per core
QF = HPC * HD              # 512
NF = QF + 2 * HD           # 768 qkv features per core
NCH = D // 128             # 32 contraction chunks
CH = 4                     # contraction chunks per h dma
NHC = NCH // CH            # 8 h dmas per t-tile
WQC = 8                    # contraction chunks per wq dma
TT = 512                   # t-tile width (matmul moving dim)
NTT = T // TT              # 4
NKT = T // 128             # 16 k-tiles
SCALING = HD ** -0.5
CAP = 30.0
BF = mybir.dt.bfloat16
F32 = mybir.dt.float32


def _emit(nc):
    hT = nc.dram_tensor("hT", [D, T], BF, kind="ExternalInput").ap()
    wq = nc.dram_tensor("wq", [D, NF], BF, kind="ExternalInput").ap()
    wo = nc.dram_tensor("wo", [QF, D], BF, kind="ExternalInput").ap()
    cc = nc.dram_tensor("cc", [HD, T], BF, kind="ExternalInput").ap()
    ss = nc.dram_tensor("ss", [HD, T], BF, kind="ExternalInput").ap()
    mk = nc.dram_tensor("mk", [4, 128, TT], BF, kind="ExternalInput").ap()
    out = nc.dram_tensor("out", [T, D], BF, kind="ExternalOutput").ap()

    with tile.TileContext(nc) as tc:
        with ExitStack() as ctx:
            wqp = ctx.enter_context(tc.tile_pool(name="wqp", bufs=1))
            wop = ctx.enter_context(tc.tile_pool(name="wop", bufs=1))
            hp = ctx.enter_context(tc.tile_pool(name="hp", bufs=8))
            cstp = ctx.enter_context(tc.tile_pool(name="cstp", bufs=1))
            seqp = ctx.enter_context(tc.tile_pool(name="seqp", bufs=1))
            rtp = ctx.enter_context(tc.tile_pool(name="rtp", bufs=2))
            stp = ctx.enter_context(tc.tile_pool(name="stp", bufs=2))
            etp = ctx.enter_context(tc.tile_pool(name="etp", bufs=3))
            eap = ctx.enter_context(tc.tile_pool(name="eap", bufs=2))
            smp = ctx.enter_context(tc.tile_pool(name="smp", bufs=2))
            obp = ctx.enter_context(tc.tile_pool(name="obp", bufs=2))
            psp = ctx.enter_context(tc.tile_pool(name="psp", bufs=1, space="PSUM"))

            # ---- resident loads: wq chunked by (i, fg-half) so the first
            # matmul only waits for 0.79 MB; stagger the rest ----
            wq_r = wq.rearrange("(i p) f -> p i f", p=128)     # i = 32 chunks
            wq4 = []
            for i in range(NCH // WQC):
                wa = wqp.tile([128, WQC, 384], BF, tag=f"wq{i}a", name=f"wq{i}a")
                wb = wqp.tile([128, WQC, 384], BF, tag=f"wq{i}b", name=f"wq{i}b")
                wq4.append((wa, wb))
            nc.gpsimd.dma_start(wq4[0][0][:], wq_r[:, 0:WQC, 0:384])
            cc_sb = cstp.tile([HD, T], BF, tag="cc")
            ss_sb = cstp.tile([HD, T], BF, tag="ss")
            mk_sb = cstp.tile([128, 4, TT], BF, tag="mk")
            # stagger the non-critical loads so they don't steal HBM
            # bandwidth from wq chunk 0 / the first h chunks
            with tc.tile_wait_until(0.005):
                nc.gpsimd.dma_start(wq4[1][0][:], wq_r[:, WQC:2 * WQC, 0:384])
            with tc.tile_wait_until(0.008):
                nc.gpsimd.dma_start(wq4[2][0][:], wq_r[:, 2 * WQC:3 * WQC, 0:384])
            with tc.tile_wait_until(0.011):
                nc.gpsimd.dma_start(wq4[3][0][:], wq_r[:, 3 * WQC:4 * WQC, 0:384])
            with tc.tile_wait_until(0.014):
                nc.gpsimd.dma_start(cc_sb[:], cc[:, :])
                nc.gpsimd.dma_start(ss_sb[:], ss[:, :])
            for i in range(NCH // WQC):
                with tc.tile_wait_until(0.022 + 0.004 * i):
                    nc.gpsimd.dma_start(
                        wq4[i][1][:], wq_r[:, i * WQC:(i + 1) * WQC, 384:NF])
            with tc.tile_wait_until(0.045):
                nc.gpsimd.dma_start(mk_sb[:], mk.rearrange("m p t -> p m t"))
            wo_r = wo.rearrange("(c p) n -> p c n", p=128)
            wo_t = []
            for j in range(2):
                w_j = wop.tile([128, 2, D], BF, tag=f"wo{j}", name=f"wo{j}")
                with tc.tile_wait_until(0.060 + 0.004 * j):
                    nc.gpsimd.dma_start(w_j[:], wo_r[:, 2 * j:2 * j + 2, :])
                wo_t.append(w_j)

            # PE warm-up: ~60 tiny matmuls fill the HAM activity window
            # while the first weight/activation DMAs are in flight, so the
            # first real matmuls run at 2.4 GHz instead of 1.2
            warm = cstp.tile([1, 64], BF, tag="warm")
            nc.gpsimd.memset(warm[:], 0.0)
            wps = psp.tile([64, 64], F32, tag="b7", name="warm_ps")
            for _ in range(60):
                nc.tensor.matmul(wps[:, :], warm[:], warm[:], start=True, stop=True)
            ident = cstp.tile([128, 128], BF, tag="id")
            make_identity(nc, ident[:])
            ones_k = cstp.tile([128, 1], BF, tag="ones_k")
            nc.gpsimd.memset(ones_k[:], 1.0)
            ones_m = cstp.tile([1, 128], BF, tag="ones_m")
            nc.gpsimd.memset(ones_m[:], 1.0)

            # per-t-tile tiles so later phases can start early
            qTt = [[seqp.tile([HD, TT], BF, tag=f"q{h}_{tt}", name=f"qT{h}_{tt}")
                    for tt in range(NTT)] for h in range(HPC)]
            kTt = [seqp.tile([HD, TT], BF, tag=f"k_{tt}", name=f"kT{tt}")
                   for tt in range(NTT)]
            vbt = [seqp.tile([128, HD], BF, tag=f"vb_{kt}", name=f"vb{kt}")
                   for kt in range(NKT)]
            atq = [[seqp.tile([HD, TT], BF, tag=f"a{h}_{qt}", name=f"at{h}_{qt}")
                    for qt in range(NTT)] for h in range(HPC)]

            # ---- phase 1: qkv projection (transposed) + rope ----
            # psum banks: b0-b2 qkv accum, b3 transposes,
            #             b4-b5 scores, b6 attn accum, b7 denom+bcast
            hT_r = hT.rearrange("(c p) t -> p c t", p=128)
            for tt in range(NTT):
                t0 = tt * TT
                hcs = []
                for hc in range(NHC):
                    h_t = hp.tile([128, CH, TT], BF, tag="h", name=f"h{tt}_{hc}")
                    with tc.tile_wait_until(
                            0.003 + 0.0012 * hc, enable=(tt == 0 and hc >= 1)):
                        nc.sync.dma_start(
                            h_t[:], hT_r[:, hc * CH:(hc + 1) * CH, t0:t0 + TT])
                    hcs.append(h_t)
                vT_t = rtp.tile([HD, TT], BF, tag="vT", name=f"vT{tt}")
                c_t = cc_sb[:, t0:t0 + TT]
                s_t = ss_sb[:, t0:t0 + TT]
                for fg in range(2):
                    ps3 = [psp.tile([128, TT], F32, tag=f"b{j}", name=f"qkv_ps{j}")
                           for j in range(3)]
                    for c in range(NCH):
                        src = hcs[c // CH][:, c % CH, :]
                        for j in range(3):
                            f = fg * 3 + j
                            nc.tensor.matmul(
                                ps3[j][:],
                                wq4[c // WQC][fg][:, c % WQC, j * 128:(j + 1) * 128],
                                src,
                                start=(c == 0),
                                stop=(c == NCH - 1),
                            )
                    for j in range(3):
                        f = fg * 3 + j
                        if f < 5:
                            dst = qTt[f][tt] if f < HPC else kTt[tt]
                            qk_sb = rtp.tile([128, TT], BF, tag="qk_sb")
                            nc.vector.tensor_copy(qk_sb[:], ps3[j][:])
                            # rotated copy: [x2; x1] via partition-swap DMA
                            rot = rtp.tile([128, TT], BF, tag="rot")
                            nc.scalar.dma_start(rot[0:HALF, :], qk_sb[HALF:128, :])
                            nc.scalar.dma_start(rot[HALF:128, :], qk_sb[0:HALF, :])
                            m1 = rtp.tile([128, TT], BF, tag="m1")
                            nc.vector.tensor_mul(m1[:], qk_sb[:], c_t)
                            m2 = rtp.tile([128, TT], BF, tag="m2")
                            nc.vector.tensor_mul(m2[:], rot[:], s_t)
                            nc.vector.tensor_add(dst[:], m1[:], m2[:])
                        else:
                            nc.vector.tensor_copy(vT_t[:], ps3[j][:])
                # transpose this t-tile's V to [t, d] blocks (DMA XBAR)
                for i in range(4):
                    kt = 4 * tt + i
                    nc.sync.dma_start(
                        vbt[kt][:], vT_t[:, i * 128:(i + 1) * 128], transpose=True)

            # ---- phase 2+3 interleaved: attention (h,qt) with o_proj
            # groups of qt-1 as PE filler while ACT catches up ----
            def oproj_group(t16, half, tail=False):
                t0o = t16 * 128
                ob = obp.tile([128, 4, TT], BF, tag="ob", name=f"ob{t16}_{half}")
                for n in range(4):
                    pls_n = psp.tile([128, TT], F32, tag=f"b{n}", name=f"o_ps{n}")
                    n0 = (half * 4 + n) * TT
                    for fc in range(HPC):
                        lhsT = atq[fc][t16 // 4][:, (t16 % 4) * 128:(t16 % 4 + 1) * 128]
                        nc.tensor.matmul(
                            pls_n[:], lhsT, wo_t[fc // 2][:, fc % 2, n0:n0 + TT],
                            start=(fc == 0), stop=(fc == HPC - 1),
                        )
                    if tail and n >= 2:
                        nc.scalar.copy(ob[:, n, :], pls_n[:])
                    else:
                        nc.vector.tensor_copy(ob[:, n, :], pls_n[:])
                outq = nc.sync if (tail and (t16 + half) % 2 == 0) else nc.gpsimd
                outq.dma_start(
                    out[t0o:t0o + 128, half * 2048:(half + 1) * 2048]
                    .rearrange("p (n t) -> p n t", n=4),
                    ob[:],
                )

            for qt in range(NTT):
                for h in range(HPC):
                    a_ps = psp.tile([HD, TT], F32, tag="b6", name="a_ps")
                    e_acc = eap.tile([128, TT], F32, tag="ea", name=f"ea{h}_{qt}")
                    nkt = 4 * qt + 4
                    for kt in range(nkt):
                        m = kt - 4 * qt
                        j0 = 128 * m if m >= 0 else 0  # skip fully-masked cols
                        s_ps = psp.tile([128, TT], F32, tag=f"b{4 + kt % 2}", name="s_ps")
                        nc.tensor.matmul(
                            s_ps[:, j0:TT], kTt[kt // 4][:, (kt % 4) * 128:(kt % 4 + 1) * 128],
                            qTt[h][qt][:, j0:TT],
                            start=True, stop=True,
                        )
                        st = stp.tile([128, TT], F32, tag="st")
                        nc.scalar.activation(
                            st[:, j0:TT], s_ps[:, j0:TT],
                            mybir.ActivationFunctionType.Tanh,
                            scale=SCALING / CAP,
                        )
                        et = etp.tile([128, TT], BF, tag="et")
                        nc.scalar.activation(
                            et[:, j0:TT], st[:, j0:TT],
                            mybir.ActivationFunctionType.Exp,
                            scale=CAP,
                        )
                        if m >= 0:
                            # causal mask: zero where k0+i > q0+j
                            nc.gpsimd.tensor_mul(
                                et[:, j0:TT], et[:, j0:TT], mk_sb[:, m, j0:TT])
                        nc.tensor.matmul(
                            a_ps[:, j0:TT], vbt[kt][:], et[:, j0:TT],
                            start=(kt == 0), stop=(kt == nkt - 1),
                        )
                        # fp32 accumulation of exp tiles for the denominator
                        if kt == 0:
                            nc.vector.tensor_copy(e_acc[:], et[:])
                        else:
                            nc.vector.tensor_add(
                                e_acc[:, j0:TT], e_acc[:, j0:TT], et[:, j0:TT])
                    e_bf = smp.tile([128, TT], BF, tag="ebf")
                    nc.vector.tensor_copy(e_bf[:], e_acc[:])
                    d_ps = psp.tile([1, TT], F32, tag="b7", name="d_ps")
                    nc.tensor.matmul(d_ps[:], ones_k[:], e_bf[:], start=True, stop=True)
                    rc = smp.tile([1, TT], F32, tag="rc")
                    nc.vector.reciprocal_approx_fast(out=rc[:], in_=d_ps[:])
                    rcb = smp.tile([1, TT], BF, tag="rcb")
                    nc.vector.tensor_copy(rcb[:], rc[:])
                    bc_ps = psp.tile([128, TT], F32, tag="b7", name="bc_ps")
                    nc.tensor.matmul(bc_ps[:], ones_m[:], rcb[:], start=True, stop=True)
                    bc_sb = smp.tile([128, TT], F32, tag="bcs")
                    nc.vector.tensor_copy(bc_sb[:], bc_ps[:])
                    nc.vector.tensor_mul(atq[h][qt][:], a_ps[:], bc_sb[:])
                    # PE filler: o_proj groups of the previous qt
                    if qt >= 1:
                        for k in range(2):
                            gi = h * 2 + k
                            oproj_group(4 * (qt - 1) + gi // 2, gi % 2,
                                        tail=(qt == 3))
            for gi in range(8):
                oproj_group(12 + gi // 2, gi % 2, tail=True)
    return nc

_CACHE = {}


def _get_nc():
    if "nc" not in _CACHE:
        nc = bacc.Bacc("TRN2", target_bir_lowering=False, debug=False)
        _emit(nc)
        nc.compile()
        _CACHE["nc"] = nc
    return _CACHE["nc"]


def _in_maps(positions, hidden_states, w_qkv, w_o):
    hidden_states = np.asarray(hidden_states, dtype=np.float32)
    w_qkv = np.asarray(w_qkv, dtype=np.float32)
    w_o = np.asarray(w_o, dtype=np.float32)
    pos = np.asarray(positions).astype(np.float64)
    bf = ml_dtypes.bfloat16

    hT = np.ascontiguousarray(hidden_states.T.astype(bf))
    inv_freq = 1.0 / (10000.0 ** (np.arange(HALF, dtype=np.float64) * 2.0 / HD))
    ang = np.outer(inv_freq, pos)                      # [64, T]
    cos = np.cos(ang).astype(np.float32)
    sin = np.sin(ang).astype(np.float32)
    cc = np.ascontiguousarray(np.concatenate([cos, cos], axis=0).astype(bf))
    ss = np.ascontiguousarray(np.concatenate([-sin, sin], axis=0).astype(bf))
    ii = np.arange(128)[:, None]
    jj = np.arange(TT)[None, :]
    mk = np.stack([(jj - ii - 128 * m >= 0) for m in range(4)]).astype(bf)

    in_maps = []
    for c in range(NCORES):
        rows = np.concatenate([
            w_qkv[QF * c:QF * (c + 1)],
            w_qkv[D + HD * c:D + HD * (c + 1)],
            w_qkv[D + HD * NCORES + HD * c:D + HD * NCORES + HD * (c + 1)],
        ], axis=0)                                      # [768, 4096]
        wq_c = np.ascontiguousarray(rows.T.astype(bf))  # [4096, 768]
        wo_c = np.ascontiguousarray(w_o[:, QF * c:QF * (c + 1)].T.astype(bf))
        in_maps.append({"hT": hT, "wq": wq_c, "wo": wo_c, "cc": cc, "ss": ss, "mk": mk})
    return in_maps


def run(positions, hidden_states, w_qkv, w_o, trace=False):
    nc = _get_nc()
    in_maps = _in_maps(positions, hidden_states, w_qkv, w_o)
    res = run_bass_kernel_spmd(nc, in_maps, list(range(NCORES)), trace=trace)
    parts = np.stack(
        [np.asarray(res.results[i]["out"]).astype(np.float32)
         for i in range(NCORES)], axis=0)
    full = parts.sum(axis=0, dtype=np.float64).astype(np.float32)
    return full, res


def kernel(positions, hidden_states, w_qkv, w_o):
    full, _ = run(positions, hidden_states, w_qkv, w_o, trace=False)
    return full
